# revision 10
# baseline (speedup 1.0000x reference)
"""Trainium2 Bass kernel: GNN message passing (child-sum TreeLSTM cell + classifier).

Math (after dead-code elimination of the reference):
  feat = emb[token_ids]                       # [N_src, D]
  x      = feat[mailbox_idx[:, -1]]           # [N_dst, D]
  h_sum  = sum_l<7 feat[mailbox_idx[:, l]]    # [N_dst, D]
  i = sigmoid(x@ix_w.T + h_sum@ih_w.T + bi)
  o = sigmoid(x@ox_w.T + h_sum@oh_w.T + bo)
  u = tanh   (x@ux_w.T + h_sum@uh_w.T + bu)
  c = i*u                                     # ch_c is all zeros -> f-branch dead
  h = o*tanh(c)
  hn = LN(h; ln2_g, ln2_b)
  logits = hn@fc_w.T + fc_b                   # [N_dst, 104]

Sharding: dst rows split across 8 cores; weights replicated.
Gather strategy: each core only references ~28k unique emb rows (of 50k vocab);
host remaps indices into a per-core compacted table (int16-safe), the bf16
table is DMA'd into SBUF once, and all 8 mailbox slots are fetched with
SBUF-source transposing dma_gather, which lands rows feature-major [D, n] --
exactly the matmul rhs layout (no on-chip transposes).  ln2 affine is folded
into the fc weights on host.
"""
import os
import sys
import math
import numpy as np

sys.path.insert(0, "/opt/trn_rl_repo")

import ml_dtypes

BF16 = ml_dtypes.bfloat16

D = 128
N_SRC = 120000
N_DST = 50000
L = 8
N_CLASSES = 104
EPS = 1e-5
N_CORES = 8

ND = N_DST // N_CORES          # 6250 dst rows per core
NDP = 6272                     # padded to 49 cols of 128
# column groups for compute: 12 groups of 4 cols (512 dst) + 1 group of 1 col
GROUPS = [(g * 4, 4) for g in range(12)] + [(48, 1)]
NIDXC = NDP * L // 16          # 3136 idx columns in the [16, .] wrapped layout

_CACHE = {}


def _idx_col_base(gi, l):
    """Column offset of (group, slot) block in the wrapped idx layout."""
    if gi < 12:
        return gi * 256 + l * 32
    return 12 * 256 + l * 8


def _build_nc(nranks, reps, nq=1):
    # nq=1: concurrent transpose-mode SBUF-source gathers on different SWDGE
    # queues intermittently corrupt 16-token write bursts on HW (verified with
    # stress_probe.py; single-queue is clean across repeated runs).
    import concourse.bass as bass
    import concourse.tile as tile
    from concourse import bacc, mybir

    fp32 = mybir.dt.float32
    bf16 = mybir.dt.bfloat16
    i16 = mybir.dt.int16
    AF = mybir.ActivationFunctionType
    ALU = mybir.AluOpType

    nc = bacc.Bacc(None, num_swdge_queues=4)

    TW = nranks * 128  # table free-dim in bf16 elements

    tabw = nc.declare_dram_parameter("tabw", [128, TW], bf16, isOutput=False)
    idxw = nc.declare_dram_parameter("idxw", [16, NIDXC], i16, isOutput=False)
    wts = nc.declare_dram_parameter("wts", [128, 6 * 128], bf16, isOutput=False)
    fcwT = nc.declare_dram_parameter("fcwT", [128, N_CLASSES], bf16, isOutput=False)
    vecs = nc.declare_dram_parameter("vecs", [128, 8], fp32, isOutput=False)
    onesm = nc.declare_dram_parameter("onesm", [128, 128], bf16, isOutput=False)
    out = nc.declare_dram_parameter("out", [N_CLASSES, NDP], fp32, isOutput=True)

    with tile.TileContext(nc) as tc:
        with (
            tc.tile_pool(name="const", bufs=1) as cpool,
            tc.tile_pool(name="gx", bufs=3) as gpool,
            tc.tile_pool(name="work", bufs=2) as wpool,
            tc.tile_pool(name="outp", bufs=2) as opool,
            tc.tile_pool(name="ps", bufs=1, space=bass.MemorySpace.PSUM) as pspool,
        ):
            reg512 = nc.gpsimd.to_reg(512)
            reg128 = nc.gpsimd.to_reg(128)

            def body():
                # ---- load constants ----
                wt = cpool.tile([128, 6 * 128], bf16, tag="wt")
                nc.sync.dma_start(out=wt[:], in_=wts[:])
                fcw = cpool.tile([128, N_CLASSES], bf16, tag="fcw")
                nc.sync.dma_start(out=fcw[:], in_=fcwT[:])
                vec = cpool.tile([128, 8], fp32, tag="vec")
                nc.sync.dma_start(out=vec[:], in_=vecs[:])
                ones_t = cpool.tile([128, 128], bf16, tag="ones")
                nc.sync.dma_start(out=ones_t[:], in_=onesm[:])
                tab = cpool.tile([128, TW], bf16, tag="tab")
                nc.sync.dma_start(out=tab[:], in_=tabw[:])
                idxt = cpool.tile([128, NIDXC], i16, tag="idx")
                for k in range(8):  # replicate [16, .] across the 8 quad-groups
                    nc.sync.dma_start(out=idxt[16 * k:16 * (k + 1), :], in_=idxw[:])

                w_ix, w_ih = wt[:, 0:128], wt[:, 128:256]
                w_ox, w_oh = wt[:, 256:384], wt[:, 384:512]
                w_ux, w_uh = wt[:, 512:640], wt[:, 640:768]
                bi, bo, bu = vec[:, 0:1], vec[:, 1:2], vec[:, 2:3]
                eps = vec[:, 3:4]
                fcb = vec[:N_CLASSES, 4:5]

                qn = 0
                for gi, (c0, ncols) in enumerate(GROUPS):
                    n = ncols * 128

                    gx = gpool.tile([128, L, 512], bf16, tag="gx")
                    for l in range(L):
                        cb = _idx_col_base(gi, l)
                        nc.gpsimd.dma_gather(
                            out_ap=gx[:, l:l + 1, :n], in_ap=tab[:],
                            idxs_ap=idxt[:, cb:cb + n // 16],
                            num_idxs=n,
                            num_idxs_reg=reg512 if n == 512 else reg128,
                            elem_size=D, transpose=True,
                            sbuf_tokens_per_rank=128,
                            sbuf_free_dim_per_rank=256,
                            queue_num=qn % nq)
                        qn += 1

                    # ---- h_sum: pairwise tree over slots 0..6 (bf16) ----
                    hs = wpool.tile([128, 512], bf16, tag="hs")
                    tmp = wpool.tile([128, 512], bf16, tag="tmp")
                    nc.vector.tensor_tensor(out=hs[:, :n], in0=gx[:, 0, :n],
                                            in1=gx[:, 1, :n], op=ALU.add)
                    nc.vector.tensor_tensor(out=tmp[:, :n], in0=gx[:, 2, :n],
                                            in1=gx[:, 3, :n], op=ALU.add)
                    nc.vector.tensor_tensor(out=hs[:, :n], in0=hs[:, :n],
                                            in1=tmp[:, :n], op=ALU.add)
                    nc.vector.tensor_tensor(out=tmp[:, :n], in0=gx[:, 4, :n],
                                            in1=gx[:, 5, :n], op=ALU.add)
                    nc.vector.tensor_tensor(out=hs[:, :n], in0=hs[:, :n],
                                            in1=tmp[:, :n], op=ALU.add)
                    nc.vector.tensor_tensor(out=hs[:, :n], in0=hs[:, :n],
                                            in1=gx[:, 6, :n], op=ALU.add)
                    xg = gx[:, 7, :n]  # self message, feature-major

                    # ---- gates: psum = Wx.T@x + Wh.T@h_sum ----
                    ps_i = pspool.tile([128, 512], fp32, tag="ps_i")
                    ps_o = pspool.tile([128, 512], fp32, tag="ps_o")
                    ps_u = pspool.tile([128, 512], fp32, tag="ps_u")
                    for ps, wx, wh in ((ps_i, w_ix, w_ih), (ps_o, w_ox, w_oh),
                                       (ps_u, w_ux, w_uh)):
                        nc.tensor.matmul(ps[:, :n], wx, xg,
                                         start=True, stop=False)
                        nc.tensor.matmul(ps[:, :n], wh, hs[:, :n],
                                         start=False, stop=True)

                    ig = wpool.tile([128, 512], bf16, tag="ig")
                    og = wpool.tile([128, 512], bf16, tag="og")
                    ug = wpool.tile([128, 512], bf16, tag="ug")
                    hg = wpool.tile([128, 512], bf16, tag="hg")
                    nc.scalar.activation(out=ig[:, :n], in_=ps_i[:, :n],
                                         func=AF.Sigmoid, bias=bi)
                    nc.scalar.activation(out=og[:, :n], in_=ps_o[:, :n],
                                         func=AF.Sigmoid, bias=bo)
                    nc.scalar.activation(out=ug[:, :n], in_=ps_u[:, :n],
                                         func=AF.Tanh, bias=bu)
                    # c = i*u (into ug), t = tanh(c) (into ig), h = o*t
                    nc.vector.tensor_tensor(out=ug[:, :n], in0=ig[:, :n],
                                            in1=ug[:, :n], op=ALU.mult)
                    nc.scalar.activation(out=ig[:, :n], in_=ug[:, :n],
                                         func=AF.Tanh)
                    nc.vector.tensor_tensor(out=hg[:, :n], in0=og[:, :n],
                                            in1=ig[:, :n], op=ALU.mult)

                    # ---- LayerNorm over features (= partitions) ----
                    sq = wpool.tile([128, 512], bf16, tag="sq")
                    nc.vector.tensor_tensor(out=sq[:, :n], in0=hg[:, :n],
                                            in1=hg[:, :n], op=ALU.mult)
                    mu_b = pspool.tile([128, 512], fp32, tag="mu_b")
                    ms_b = pspool.tile([128, 512], fp32, tag="ms_b")
                    nc.tensor.matmul(mu_b[:, :n], ones_t[:], hg[:, :n],
                                     start=True, stop=True)
                    nc.tensor.matmul(ms_b[:, :n], ones_t[:], sq[:, :n],
                                     start=True, stop=True)
                    var = wpool.tile([128, 512], fp32, tag="var")
                    nc.scalar.activation(out=var[:, :n], in_=mu_b[:, :n],
                                         func=AF.Square)
                    nc.vector.tensor_tensor(out=var[:, :n], in0=ms_b[:, :n],
                                            in1=var[:, :n], op=ALU.subtract)
                    # rstd = 1/sqrt(var + eps)
                    nc.scalar.activation(out=var[:, :n], in_=var[:, :n],
                                         func=AF.Sqrt, bias=eps)
                    nc.vector.reciprocal(out=var[:, :n], in_=var[:, :n])
                    # hn = (h - mu) * rstd   (ln2 affine folded into fc)
                    nc.vector.tensor_tensor(out=hg[:, :n], in0=hg[:, :n],
                                            in1=mu_b[:, :n], op=ALU.subtract)
                    nc.vector.tensor_tensor(out=hg[:, :n], in0=hg[:, :n],
                                            in1=var[:, :n], op=ALU.mult)

                    # ---- fc head: logits.T [104, n] ----
                    fcp = pspool.tile([N_CLASSES, 512], fp32, tag="fcp")
                    nc.tensor.matmul(fcp[:, :n], fcw[:], hg[:, :n],
                                     start=True, stop=True)
                    lg = opool.tile([N_CLASSES, 512], fp32, tag="lg")
                    nc.scalar.activation(out=lg[:, :n], in_=fcp[:, :n],
                                         func=AF.Identity, bias=fcb)
                    nc.sync.dma_start(out=out[:, c0 * 128: c0 * 128 + n],
                                      in_=lg[:, :n])

            if reps == 1:
                body()
            else:
                with tc.For_i(0, reps):
                    body()

    # Align each gather's SWDGE queue with its Tile-assigned DMASW sem lane
    # (sim/HW require a consistent sem<->queue pairing).
    from concourse import bass_isa
    DMASW0 = 11
    for b in nc.m.functions[0].blocks:
        for inst in b.instructions:
            if isinstance(inst, mybir.InstDMAGatherAnt):
                inst.queue_num = (inst.bass_scheduled_proc - DMASW0) % nq
    # The For_i reset/skip blocks bulk-adjust all 8 DMASW sems with a single
    # queue-0 InstIncSwdgeSem; each sem is locked to the queue (lane % 4) its
    # gathers use, so split the adjustment into one instruction per queue.
    for b in nc.m.functions[0].blocks:
        new_insts = []
        for inst in b.instructions:
            new_insts.append(inst)
            if not (isinstance(inst, bass_isa.InstIncSwdgeSem)
                    and inst._mode in ("add", "sub")):
                continue
            names = list(inst._sem_names)
            vals = list(inst._sem_values)
            lanes = [i for i, (nm, v) in enumerate(zip(names, vals))
                     if v and nm.startswith("DMASW")]
            qs = sorted({i % 4 for i in lanes})
            if len(qs) <= 1:
                if lanes:
                    inst.queue_num = qs[0]
                continue

            def masked(q):
                return [v if (i % 4 == q and i in lanes) else 0
                        for i, v in enumerate(vals)]

            inst._sem_values = masked(qs[0])
            inst.queue_num = qs[0]
            for q in qs[1:]:
                clone = bass_isa.InstIncSwdgeSem(
                    _sem_id_base=inst._sem_id_base,
                    _sem_values=masked(q),
                    _sem_names=names,
                    queue_num=q,
                    _mode=inst._mode,
                    name=nc.get_next_instruction_name(),
                    engine=inst.engine,
                )
                nc.register_instruction(clone)
                new_insts.append(clone)
        b.instructions[:] = new_insts
    nc.finalize()
    return nc


def get_nc(nranks, reps=1):
    key = ("nc", nranks, reps)
    if key not in _CACHE:
        _CACHE[key] = _build_nc(nranks, reps)
    return _CACHE[key]


def _prep_host(inputs):
    token_ids = np.asarray(inputs["token_ids"]).astype(np.int64)
    mailbox_idx = np.asarray(inputs["mailbox_idx"]).astype(np.int64)
    emb = np.asarray(inputs["emb"], dtype=np.float32)

    idx2 = token_ids[mailbox_idx]  # [N_DST, L]

    uniqs, invs = [], []
    for c in range(N_CORES):
        rows = idx2[c * ND:(c + 1) * ND]
        uniq, inv = np.unique(rows, return_inverse=True)
        assert uniq.size <= 32767, f"core {c}: {uniq.size} unique rows > int16"
        uniqs.append(uniq)
        invs.append(inv.reshape(ND, L))
    nranks = (max(u.size for u in uniqs) + 127) // 128

    # weights: ln2 affine folded into the fc head
    ln2_g = np.asarray(inputs["ln2_g"], np.float32)
    ln2_b = np.asarray(inputs["ln2_b"], np.float32)
    fc_w = np.asarray(inputs["fc_w"], np.float32)
    wts = np.concatenate(
        [np.ascontiguousarray(np.asarray(inputs[k], np.float32).T) for k in
         ("ix_w", "ih_w", "ox_w", "oh_w", "ux_w", "uh_w")],
        axis=1).astype(BF16)  # [128, 768]
    fcwT = np.ascontiguousarray((fc_w * ln2_g[None, :]).T).astype(BF16)
    vecs = np.zeros((128, 8), np.float32)
    vecs[:, 0] = np.asarray(inputs["ix_b"]) + np.asarray(inputs["ih_b"])
    vecs[:, 1] = np.asarray(inputs["ox_b"]) + np.asarray(inputs["oh_b"])
    vecs[:, 2] = np.asarray(inputs["ux_b"]) + np.asarray(inputs["uh_b"])
    vecs[:, 3] = EPS
    vecs[:N_CLASSES, 4] = np.asarray(inputs["fc_b"], np.float32) + fc_w @ ln2_b
    onesm = np.full((128, 128), 1.0 / D, np.float32).astype(BF16)

    shared = dict(wts=wts, fcwT=fcwT, vecs=vecs, onesm=onesm)
    in_maps = []
    for c in range(N_CORES):
        uniq, inv = uniqs[c], invs[c]
        tw = nranks * 128
        tabf = np.zeros((tw, D), np.float32)
        tabf[:uniq.size] = emb[uniq]
        tabw = np.ascontiguousarray(
            tabf.reshape(nranks, 128, D).transpose(1, 0, 2).reshape(128, tw)
        ).astype(BF16)

        inv2 = np.zeros((NDP, L), np.int64)
        inv2[:ND] = inv
        Y = inv2[:6144].reshape(12, 32, 16, L).transpose(2, 0, 3, 1).reshape(16, -1)
        Z = inv2[6144:].reshape(8, 16, L).transpose(1, 2, 0).reshape(16, -1)
        idxw = np.concatenate([Y, Z], axis=1).astype(np.int16)  # [16, 3136]

        m = dict(shared)
        m["tabw"] = tabw
        m["idxw"] = idxw
        in_maps.append(m)
    return in_maps, nranks


def kernel(**inputs):
    from concourse.bass_utils import run_bass_kernel_spmd

    in_maps, nranks = _prep_host(inputs)
    _CACHE["in_maps"] = in_maps
    _CACHE["nranks"] = nranks
    nc = get_nc(nranks, reps=1)

    res = run_bass_kernel_spmd(nc, in_maps, list(range(N_CORES)))
    _CACHE["last_results"] = res

    out = np.empty((N_DST, N_CLASSES), np.float32)
    for c in range(N_CORES):
        out[c * ND:(c + 1) * ND] = res.results[c]["out"][:, :ND].T
    return out


# ---------------------------------------------------------------------------
# Timing: HW exec time of one kernel execution, measured as the marginal cost
# per iteration of an on-device For_i reps-loop (cancels the axon RPC
# dispatch overhead which would otherwise swamp the kernel by ~500x).
# Inputs are staged on device once; the same jitted executable is reused.
# ---------------------------------------------------------------------------

def _make_runner(nc, in_maps):
    import jax
    import numpy as np
    from jax.sharding import Mesh, PartitionSpec, NamedSharding
    from jax.experimental.shard_map import shard_map
    from concourse import bass2jax, mybir
    bass2jax.install_neuronx_cc_hook()

    assert nc.partition_id_tensor is None
    in_names, out_names, out_avals, zero_outs = [], [], [], []
    for alloc in nc.m.functions[0].allocations:
        if not isinstance(alloc, mybir.MemoryLocationSet):
            continue
        name = alloc.memorylocations[0].name
        if alloc.kind == "ExternalInput":
            in_names.append(name)
        elif alloc.kind == "ExternalOutput":
            out_names.append(name)
            shape = tuple(alloc.tensor_shape)
            dtype = mybir.dt.np(alloc.dtype)
            out_avals.append(jax.core.ShapedArray(shape, dtype))
            zero_outs.append(np.zeros(shape, dtype))
    n_params = len(in_names)
    n_outs = len(out_avals)
    all_names = in_names + out_names

    def _body(*args):
        outs = bass2jax._bass_exec_p.bind(
            *args,
            out_avals=tuple(out_avals),
            in_names=tuple(all_names),
            out_names=tuple(out_names),
            lowering_input_output_aliases=(),
            sim_require_finite=True,
            sim_require_nnan=True,
            nc=nc,
        )
        return tuple(outs)

    devices = jax.devices()[:N_CORES]
    mesh = Mesh(np.asarray(devices), ("core",))
    spec = PartitionSpec("core")
    sharded = jax.jit(
        shard_map(_body, mesh=mesh, in_specs=(spec,) * (n_params + n_outs),
                  out_specs=(spec,) * n_outs, check_rep=False),
        donate_argnums=tuple(range(n_params, n_params + n_outs)),
        keep_unused=True,
    )
    sharding = NamedSharding(mesh, spec)
    dkey = ("dev_in", tuple(in_names))
    if dkey not in _CACHE:
        dev_in = [
            jax.device_put(
                np.concatenate([np.asarray(in_maps[c][nm])
                                for c in range(N_CORES)], axis=0), sharding)
            for nm in in_names
        ]
        for a in dev_in:
            a.block_until_ready()
        _CACHE[dkey] = dev_in
    dev_in = _CACHE[dkey]

    def run():
        zs = [jax.device_put(
            np.zeros((N_CORES * z.shape[0], *z.shape[1:]), z.dtype), sharding)
            for z in zero_outs]
        for z in zs:
            z.block_until_ready()
        import time
        t0 = time.perf_counter()
        outs = sharded(*dev_in, *zs)
        jax.block_until_ready(outs)
        return time.perf_counter() - t0, outs

    return run


def measure_hw_exec_ns(reps=1024, n_meas=5, verbose=True):
    """Marginal per-iteration HW time: (t[reps] - t[1]) / (reps - 1)."""
    in_maps, nranks = _CACHE["in_maps"], _CACHE["nranks"]
    if ("runner", 1) not in _CACHE:
        _CACHE[("runner", 1)] = _make_runner(get_nc(nranks, 1), in_maps)
    if ("runner", reps) not in _CACHE:
        _CACHE[("runner", reps)] = _make_runner(get_nc(nranks, reps), in_maps)
    r1, rk = _CACHE[("runner", 1)], _CACHE[("runner", reps)]
    # warm both executables; cross-check the staged-input path against the
    # correctness run
    _, outs1 = r1()
    ref_res = _CACHE.get("last_results")
    if ref_res is not None:
        got = np.asarray(outs1[0]).reshape(N_CORES, N_CLASSES, NDP)
        for c in range(N_CORES):
            assert np.allclose(got[c], ref_res.results[c]["out"],
                               rtol=1e-3, atol=1e-3), f"runner mismatch core {c}"
    _, outsk = rk()
    gotk = np.asarray(outsk[0]).reshape(N_CORES, N_CLASSES, NDP)
    assert np.allclose(gotk, np.asarray(outs1[0]).reshape(gotk.shape),
                       rtol=1e-3, atol=1e-3), "reps-loop output mismatch"
    t1s, tks = [], []
    for _ in range(n_meas):
        t1s.append(r1()[0])
        tks.append(rk()[0])
    t1s.sort(); tks.sort()
    med1 = t1s[len(t1s) // 2]
    medk = tks[len(tks) // 2]
    if verbose:
        print(f"t[1] runs: {[f'{t*1e3:.1f}ms' for t in t1s]}")
        print(f"t[{reps}] runs: {[f'{t*1e3:.1f}ms' for t in tks]}")
    return (medk - med1) / (reps - 1) * 1e9


# revision 11
# speedup vs baseline: 3377.1726x; 3377.1726x over previous
"""Trainium2 Bass kernel: GNN message passing (child-sum TreeLSTM cell + classifier).

Math (after dead-code elimination of the reference):
  feat = emb[token_ids]                       # [N_src, D]
  x      = feat[mailbox_idx[:, -1]]           # [N_dst, D]
  h_sum  = sum_l<7 feat[mailbox_idx[:, l]]    # [N_dst, D]
  i = sigmoid(x@ix_w.T + h_sum@ih_w.T + bi)
  o = sigmoid(x@ox_w.T + h_sum@oh_w.T + bo)
  u = tanh   (x@ux_w.T + h_sum@uh_w.T + bu)
  c = i*u                                     # ch_c is all zeros -> f-branch dead
  h = o*tanh(c)
  hn = LN(h; ln2_g, ln2_b)
  logits = hn@fc_w.T + fc_b                   # [N_dst, 104]

Sharding: dst rows split across 8 cores; weights replicated.
Gather strategy: each core only references ~28k unique emb rows (of 50k vocab);
host remaps indices into a per-core compacted table (int16-safe), the bf16
table is DMA'd into SBUF once, and all 8 mailbox slots are fetched with
SBUF-source transposing dma_gather, which lands rows feature-major [D, n] --
exactly the matmul rhs layout (no on-chip transposes).  ln2 affine is folded
into the fc weights on host.
"""
import os
import sys
import math
import numpy as np

sys.path.insert(0, "/opt/trn_rl_repo")

import ml_dtypes

BF16 = ml_dtypes.bfloat16

D = 128
N_SRC = 120000
N_DST = 50000
L = 8
N_CLASSES = 104
EPS = 1e-5
N_CORES = 8

ND = N_DST // N_CORES          # 6250 dst rows per core
NDP = 6272                     # padded to 49 cols of 128
# column groups for compute: 12 groups of 4 cols (512 dst) + 1 group of 1 col
GROUPS = [(g * 4, 4) for g in range(12)] + [(48, 1)]
NIDXC = NDP * L // 16          # 3136 idx columns in the [16, .] wrapped layout

_CACHE = {}


def _idx_col_base(gi, l):
    """Column offset of (group, slot) block in the wrapped idx layout."""
    if gi < 12:
        return gi * 256 + l * 32
    return 12 * 256 + l * 8


def _build_nc(nranks, reps, nq=1):
    # nq=1: concurrent transpose-mode SBUF-source gathers on different SWDGE
    # queues intermittently corrupt 16-token write bursts on HW (verified with
    # stress_probe.py; single-queue is clean across repeated runs).
    import concourse.bass as bass
    import concourse.tile as tile
    from concourse import bacc, mybir

    fp32 = mybir.dt.float32
    bf16 = mybir.dt.bfloat16
    i16 = mybir.dt.int16
    AF = mybir.ActivationFunctionType
    ALU = mybir.AluOpType

    nc = bacc.Bacc(None, num_swdge_queues=4)

    TW = nranks * 128  # table free-dim in bf16 elements

    tabw = nc.declare_dram_parameter("tabw", [128, TW], bf16, isOutput=False)
    idxw = nc.declare_dram_parameter("idxw", [16, NIDXC], i16, isOutput=False)
    wts = nc.declare_dram_parameter("wts", [128, 6 * 128], bf16, isOutput=False)
    fcwT = nc.declare_dram_parameter("fcwT", [128, N_CLASSES], bf16, isOutput=False)
    vecs = nc.declare_dram_parameter("vecs", [128, 8], fp32, isOutput=False)
    onesm = nc.declare_dram_parameter("onesm", [128, 128], bf16, isOutput=False)
    out = nc.declare_dram_parameter("out", [N_CLASSES, NDP], fp32, isOutput=True)

    with tile.TileContext(nc) as tc:
        with (
            tc.tile_pool(name="const", bufs=1) as cpool,
            tc.tile_pool(name="gx", bufs=3) as gpool,
            tc.tile_pool(name="work", bufs=2) as wpool,
            tc.tile_pool(name="outp", bufs=2) as opool,
            tc.tile_pool(name="ps", bufs=1, space=bass.MemorySpace.PSUM) as pspool,
        ):
            reg512 = nc.gpsimd.to_reg(512)
            reg128 = nc.gpsimd.to_reg(128)

            def body():
                # ---- load constants ----
                wt = cpool.tile([128, 6 * 128], bf16, tag="wt")
                nc.sync.dma_start(out=wt[:], in_=wts[:])
                fcw = cpool.tile([128, N_CLASSES], bf16, tag="fcw")
                nc.sync.dma_start(out=fcw[:], in_=fcwT[:])
                vec = cpool.tile([128, 8], fp32, tag="vec")
                nc.sync.dma_start(out=vec[:], in_=vecs[:])
                ones_t = cpool.tile([128, 128], bf16, tag="ones")
                nc.sync.dma_start(out=ones_t[:], in_=onesm[:])
                tab = cpool.tile([128, TW], bf16, tag="tab")
                nc.sync.dma_start(out=tab[:], in_=tabw[:])
                idxt = cpool.tile([128, NIDXC], i16, tag="idx")
                for k in range(8):  # replicate [16, .] across the 8 quad-groups
                    nc.sync.dma_start(out=idxt[16 * k:16 * (k + 1), :], in_=idxw[:])

                w_ix, w_ih = wt[:, 0:128], wt[:, 128:256]
                w_ox, w_oh = wt[:, 256:384], wt[:, 384:512]
                w_ux, w_uh = wt[:, 512:640], wt[:, 640:768]
                bi, bo, bu = vec[:, 0:1], vec[:, 1:2], vec[:, 2:3]
                eps = vec[:, 3:4]
                fcb = vec[:N_CLASSES, 4:5]

                qn = 0
                for gi, (c0, ncols) in enumerate(GROUPS):
                    n = ncols * 128

                    gx = gpool.tile([128, L, 512], bf16, tag="gx")
                    for l in range(L):
                        cb = _idx_col_base(gi, l)
                        nc.gpsimd.dma_gather(
                            out_ap=gx[:, l:l + 1, :n], in_ap=tab[:],
                            idxs_ap=idxt[:, cb:cb + n // 16],
                            num_idxs=n,
                            num_idxs_reg=reg512 if n == 512 else reg128,
                            elem_size=D, transpose=True,
                            sbuf_tokens_per_rank=128,
                            sbuf_free_dim_per_rank=256,
                            queue_num=qn % nq)
                        qn += 1

                    # ---- h_sum: pairwise tree over slots 0..6 (bf16) ----
                    hs = wpool.tile([128, 512], bf16, tag="hs")
                    tmp = wpool.tile([128, 512], bf16, tag="tmp")
                    nc.vector.tensor_tensor(out=hs[:, :n], in0=gx[:, 0, :n],
                                            in1=gx[:, 1, :n], op=ALU.add)
                    nc.vector.tensor_tensor(out=tmp[:, :n], in0=gx[:, 2, :n],
                                            in1=gx[:, 3, :n], op=ALU.add)
                    nc.vector.tensor_tensor(out=hs[:, :n], in0=hs[:, :n],
                                            in1=tmp[:, :n], op=ALU.add)
                    nc.vector.tensor_tensor(out=tmp[:, :n], in0=gx[:, 4, :n],
                                            in1=gx[:, 5, :n], op=ALU.add)
                    nc.vector.tensor_tensor(out=hs[:, :n], in0=hs[:, :n],
                                            in1=tmp[:, :n], op=ALU.add)
                    nc.vector.tensor_tensor(out=hs[:, :n], in0=hs[:, :n],
                                            in1=gx[:, 6, :n], op=ALU.add)
                    xg = gx[:, 7, :n]  # self message, feature-major

                    # ---- gates: psum = Wx.T@x + Wh.T@h_sum ----
                    ps_i = pspool.tile([128, 512], fp32, tag="ps_i")
                    ps_o = pspool.tile([128, 512], fp32, tag="ps_o")
                    ps_u = pspool.tile([128, 512], fp32, tag="ps_u")
                    for ps, wx, wh in ((ps_i, w_ix, w_ih), (ps_o, w_ox, w_oh),
                                       (ps_u, w_ux, w_uh)):
                        nc.tensor.matmul(ps[:, :n], wx, xg,
                                         start=True, stop=False)
                        nc.tensor.matmul(ps[:, :n], wh, hs[:, :n],
                                         start=False, stop=True)

                    ig = wpool.tile([128, 512], bf16, tag="ig")
                    og = wpool.tile([128, 512], bf16, tag="og")
                    ug = wpool.tile([128, 512], bf16, tag="ug")
                    hg = wpool.tile([128, 512], bf16, tag="hg")
                    nc.scalar.activation(out=ig[:, :n], in_=ps_i[:, :n],
                                         func=AF.Sigmoid, bias=bi)
                    nc.scalar.activation(out=og[:, :n], in_=ps_o[:, :n],
                                         func=AF.Sigmoid, bias=bo)
                    nc.scalar.activation(out=ug[:, :n], in_=ps_u[:, :n],
                                         func=AF.Tanh, bias=bu)
                    # c = i*u (into ug), t = tanh(c) (into ig), h = o*t
                    nc.vector.tensor_tensor(out=ug[:, :n], in0=ig[:, :n],
                                            in1=ug[:, :n], op=ALU.mult)
                    nc.scalar.activation(out=ig[:, :n], in_=ug[:, :n],
                                         func=AF.Tanh)
                    nc.vector.tensor_tensor(out=hg[:, :n], in0=og[:, :n],
                                            in1=ig[:, :n], op=ALU.mult)

                    # ---- LayerNorm over features (= partitions) ----
                    sq = wpool.tile([128, 512], bf16, tag="sq")
                    nc.vector.tensor_tensor(out=sq[:, :n], in0=hg[:, :n],
                                            in1=hg[:, :n], op=ALU.mult)
                    mu_b = pspool.tile([128, 512], fp32, tag="mu_b")
                    ms_b = pspool.tile([128, 512], fp32, tag="ms_b")
                    nc.tensor.matmul(mu_b[:, :n], ones_t[:], hg[:, :n],
                                     start=True, stop=True)
                    nc.tensor.matmul(ms_b[:, :n], ones_t[:], sq[:, :n],
                                     start=True, stop=True)
                    var = wpool.tile([128, 512], fp32, tag="var")
                    nc.scalar.activation(out=var[:, :n], in_=mu_b[:, :n],
                                         func=AF.Square)
                    nc.vector.tensor_tensor(out=var[:, :n], in0=ms_b[:, :n],
                                            in1=var[:, :n], op=ALU.subtract)
                    # rstd = 1/sqrt(var + eps)
                    nc.scalar.activation(out=var[:, :n], in_=var[:, :n],
                                         func=AF.Sqrt, bias=eps)
                    nc.vector.reciprocal(out=var[:, :n], in_=var[:, :n])
                    # hn = (h - mu) * rstd   (ln2 affine folded into fc)
                    nc.vector.tensor_tensor(out=hg[:, :n], in0=hg[:, :n],
                                            in1=mu_b[:, :n], op=ALU.subtract)
                    nc.vector.tensor_tensor(out=hg[:, :n], in0=hg[:, :n],
                                            in1=var[:, :n], op=ALU.mult)

                    # ---- fc head: logits.T [104, n] ----
                    fcp = pspool.tile([N_CLASSES, 512], fp32, tag="fcp")
                    nc.tensor.matmul(fcp[:, :n], fcw[:], hg[:, :n],
                                     start=True, stop=True)
                    lg = opool.tile([N_CLASSES, 512], fp32, tag="lg")
                    nc.scalar.activation(out=lg[:, :n], in_=fcp[:, :n],
                                         func=AF.Identity, bias=fcb)
                    nc.sync.dma_start(out=out[:, c0 * 128: c0 * 128 + n],
                                      in_=lg[:, :n])

            if reps == 1:
                body()
            else:
                with tc.For_i(0, reps):
                    body()

    # Align each gather's SWDGE queue with its Tile-assigned DMASW sem lane
    # (sim/HW require a consistent sem<->queue pairing).
    from concourse import bass_isa
    DMASW0 = 11
    for b in nc.m.functions[0].blocks:
        for inst in b.instructions:
            if isinstance(inst, mybir.InstDMAGatherAnt):
                inst.queue_num = (inst.bass_scheduled_proc - DMASW0) % nq
    # The For_i reset/skip blocks bulk-adjust all 8 DMASW sems with a single
    # queue-0 InstIncSwdgeSem; each sem is locked to the queue (lane % 4) its
    # gathers use, so split the adjustment into one instruction per queue.
    for b in nc.m.functions[0].blocks:
        new_insts = []
        for inst in b.instructions:
            new_insts.append(inst)
            if not (isinstance(inst, bass_isa.InstIncSwdgeSem)
                    and inst._mode in ("add", "sub")):
                continue
            names = list(inst._sem_names)
            vals = list(inst._sem_values)
            lanes = [i for i, (nm, v) in enumerate(zip(names, vals))
                     if v and nm.startswith("DMASW")]
            qs = sorted({i % 4 for i in lanes})
            if len(qs) <= 1:
                if lanes:
                    inst.queue_num = qs[0]
                continue

            def masked(q):
                return [v if (i % 4 == q and i in lanes) else 0
                        for i, v in enumerate(vals)]

            inst._sem_values = masked(qs[0])
            inst.queue_num = qs[0]
            for q in qs[1:]:
                clone = bass_isa.InstIncSwdgeSem(
                    _sem_id_base=inst._sem_id_base,
                    _sem_values=masked(q),
                    _sem_names=names,
                    queue_num=q,
                    _mode=inst._mode,
                    name=nc.get_next_instruction_name(),
                    engine=inst.engine,
                )
                nc.register_instruction(clone)
                new_insts.append(clone)
        b.instructions[:] = new_insts
    nc.finalize()
    return nc


def get_nc(nranks, reps=1):
    key = ("nc", nranks, reps)
    if key not in _CACHE:
        _CACHE[key] = _build_nc(nranks, reps)
    return _CACHE[key]


def _prep_host(inputs):
    token_ids = np.asarray(inputs["token_ids"]).astype(np.int64)
    mailbox_idx = np.asarray(inputs["mailbox_idx"]).astype(np.int64)
    emb = np.asarray(inputs["emb"], dtype=np.float32)

    idx2 = token_ids[mailbox_idx]  # [N_DST, L]

    uniqs, invs = [], []
    for c in range(N_CORES):
        rows = idx2[c * ND:(c + 1) * ND]
        uniq, inv = np.unique(rows, return_inverse=True)
        assert uniq.size <= 32767, f"core {c}: {uniq.size} unique rows > int16"
        uniqs.append(uniq)
        invs.append(inv.reshape(ND, L))
    nranks = (max(u.size for u in uniqs) + 127) // 128

    # weights: ln2 affine folded into the fc head
    ln2_g = np.asarray(inputs["ln2_g"], np.float32)
    ln2_b = np.asarray(inputs["ln2_b"], np.float32)
    fc_w = np.asarray(inputs["fc_w"], np.float32)
    wts = np.concatenate(
        [np.ascontiguousarray(np.asarray(inputs[k], np.float32).T) for k in
         ("ix_w", "ih_w", "ox_w", "oh_w", "ux_w", "uh_w")],
        axis=1).astype(BF16)  # [128, 768]
    fcwT = np.ascontiguousarray((fc_w * ln2_g[None, :]).T).astype(BF16)
    vecs = np.zeros((128, 8), np.float32)
    vecs[:, 0] = np.asarray(inputs["ix_b"]) + np.asarray(inputs["ih_b"])
    vecs[:, 1] = np.asarray(inputs["ox_b"]) + np.asarray(inputs["oh_b"])
    vecs[:, 2] = np.asarray(inputs["ux_b"]) + np.asarray(inputs["uh_b"])
    vecs[:, 3] = EPS
    vecs[:N_CLASSES, 4] = np.asarray(inputs["fc_b"], np.float32) + fc_w @ ln2_b
    onesm = np.full((128, 128), 1.0 / D, np.float32).astype(BF16)

    shared = dict(wts=wts, fcwT=fcwT, vecs=vecs, onesm=onesm)
    in_maps = []
    for c in range(N_CORES):
        uniq, inv = uniqs[c], invs[c]
        tw = nranks * 128
        tabf = np.zeros((tw, D), np.float32)
        tabf[:uniq.size] = emb[uniq]
        tabw = np.ascontiguousarray(
            tabf.reshape(nranks, 128, D).transpose(1, 0, 2).reshape(128, tw)
        ).astype(BF16)

        inv2 = np.zeros((NDP, L), np.int64)
        inv2[:ND] = inv
        Y = inv2[:6144].reshape(12, 32, 16, L).transpose(2, 0, 3, 1).reshape(16, -1)
        Z = inv2[6144:].reshape(8, 16, L).transpose(1, 2, 0).reshape(16, -1)
        idxw = np.concatenate([Y, Z], axis=1).astype(np.int16)  # [16, 3136]

        m = dict(shared)
        m["tabw"] = tabw
        m["idxw"] = idxw
        in_maps.append(m)
    return in_maps, nranks


def kernel(**inputs):
    from concourse.bass_utils import run_bass_kernel_spmd

    in_maps, nranks = _prep_host(inputs)
    _CACHE["in_maps"] = in_maps
    _CACHE["nranks"] = nranks
    nc = get_nc(nranks, reps=1)

    res = run_bass_kernel_spmd(nc, in_maps, list(range(N_CORES)))
    _CACHE["last_results"] = res

    out = np.empty((N_DST, N_CLASSES), np.float32)
    for c in range(N_CORES):
        out[c * ND:(c + 1) * ND] = res.results[c]["out"][:, :ND].T
    return out


# ---------------------------------------------------------------------------
# Timing: HW exec time of one kernel execution, measured as the marginal cost
# per iteration of an on-device For_i reps-loop (cancels the axon RPC
# dispatch overhead which would otherwise swamp the kernel by ~500x).
# Inputs are staged on device once; the same jitted executable is reused.
# ---------------------------------------------------------------------------

def _make_runner(nc, in_maps):
    import jax
    import numpy as np
    from jax.sharding import Mesh, PartitionSpec, NamedSharding
    from jax.experimental.shard_map import shard_map
    from concourse import bass2jax, mybir
    bass2jax.install_neuronx_cc_hook()

    partition_name = (nc.partition_id_tensor.name
                      if nc.partition_id_tensor else None)
    in_names, out_names, out_avals, zero_outs = [], [], [], []
    for alloc in nc.m.functions[0].allocations:
        if not isinstance(alloc, mybir.MemoryLocationSet):
            continue
        name = alloc.memorylocations[0].name
        if alloc.kind == "ExternalInput":
            if name != partition_name:
                in_names.append(name)
        elif alloc.kind == "ExternalOutput":
            out_names.append(name)
            shape = tuple(alloc.tensor_shape)
            dtype = mybir.dt.np(alloc.dtype)
            out_avals.append(jax.core.ShapedArray(shape, dtype))
            zero_outs.append(np.zeros(shape, dtype))
    n_params = len(in_names)
    n_outs = len(out_avals)
    all_names = in_names + out_names
    if partition_name is not None:
        all_names = all_names + [partition_name]

    def _body(*args):
        operands = list(args)
        if partition_name is not None:
            operands.append(bass2jax.partition_id_tensor())
        outs = bass2jax._bass_exec_p.bind(
            *operands,
            out_avals=tuple(out_avals),
            in_names=tuple(all_names),
            out_names=tuple(out_names),
            lowering_input_output_aliases=(),
            sim_require_finite=True,
            sim_require_nnan=True,
            nc=nc,
        )
        return tuple(outs)

    devices = jax.devices()[:N_CORES]
    mesh = Mesh(np.asarray(devices), ("core",))
    spec = PartitionSpec("core")
    sharded = jax.jit(
        shard_map(_body, mesh=mesh, in_specs=(spec,) * (n_params + n_outs),
                  out_specs=(spec,) * n_outs, check_rep=False),
        donate_argnums=tuple(range(n_params, n_params + n_outs)),
        keep_unused=True,
    )
    sharding = NamedSharding(mesh, spec)
    dkey = ("dev_in", tuple(in_names))
    if dkey not in _CACHE:
        dev_in = [
            jax.device_put(
                np.concatenate([np.asarray(in_maps[c][nm])
                                for c in range(N_CORES)], axis=0), sharding)
            for nm in in_names
        ]
        for a in dev_in:
            a.block_until_ready()
        _CACHE[dkey] = dev_in
    dev_in = _CACHE[dkey]

    def run():
        zs = [jax.device_put(
            np.zeros((N_CORES * z.shape[0], *z.shape[1:]), z.dtype), sharding)
            for z in zero_outs]
        for z in zs:
            z.block_until_ready()
        import time
        t0 = time.perf_counter()
        outs = sharded(*dev_in, *zs)
        jax.block_until_ready(outs)
        return time.perf_counter() - t0, outs

    return run


def measure_hw_exec_ns(reps=1024, n_meas=5, verbose=True):
    """Marginal per-iteration HW time: (t[reps] - t[1]) / (reps - 1)."""
    in_maps, nranks = _CACHE["in_maps"], _CACHE["nranks"]
    if ("runner", 1) not in _CACHE:
        _CACHE[("runner", 1)] = _make_runner(get_nc(nranks, 1), in_maps)
    if ("runner", reps) not in _CACHE:
        _CACHE[("runner", reps)] = _make_runner(get_nc(nranks, reps), in_maps)
    r1, rk = _CACHE[("runner", 1)], _CACHE[("runner", reps)]
    # warm both executables; cross-check the staged-input path against the
    # correctness run
    _, outs1 = r1()
    ref_res = _CACHE.get("last_results")
    if ref_res is not None:
        got = np.asarray(outs1[0]).reshape(N_CORES, N_CLASSES, NDP)
        for c in range(N_CORES):
            assert np.allclose(got[c], ref_res.results[c]["out"],
                               rtol=1e-3, atol=1e-3), f"runner mismatch core {c}"
    _, outsk = rk()
    gotk = np.asarray(outsk[0]).reshape(N_CORES, N_CLASSES, NDP)
    assert np.allclose(gotk, np.asarray(outs1[0]).reshape(gotk.shape),
                       rtol=1e-3, atol=1e-3), "reps-loop output mismatch"
    t1s, tks = [], []
    for _ in range(n_meas):
        t1s.append(r1()[0])
        tks.append(rk()[0])
    t1s.sort(); tks.sort()
    med1 = t1s[len(t1s) // 2]
    medk = tks[len(tks) // 2]
    if verbose:
        print(f"t[1] runs: {[f'{t*1e3:.1f}ms' for t in t1s]}")
        print(f"t[{reps}] runs: {[f'{t*1e3:.1f}ms' for t in tks]}")
    return (medk - med1) / (reps - 1) * 1e9


# revision 15
# speedup vs baseline: 8048.3566x; 2.3832x over previous
"""Trainium2 Bass kernel: GNN message passing (child-sum TreeLSTM cell + classifier).

Math (after dead-code elimination of the reference):
  feat = emb[token_ids]                       # [N_src, D]
  x      = feat[mailbox_idx[:, -1]]           # [N_dst, D]
  h_sum  = sum_l<7 feat[mailbox_idx[:, l]]    # [N_dst, D]
  i = sigmoid(x@ix_w.T + h_sum@ih_w.T + bi)
  o = sigmoid(x@ox_w.T + h_sum@oh_w.T + bo)
  u = tanh   (x@ux_w.T + h_sum@uh_w.T + bu)
  c = i*u                                     # ch_c is all zeros -> f-branch dead
  h = o*tanh(c)
  hn = LN(h; ln2_g, ln2_b)
  logits = hn@fc_w.T + fc_b                   # [N_dst, 104]

Sharding: dst rows split across 8 cores; weights replicated.
Gather strategy: each core only references ~28k unique emb rows (of 50k vocab);
host remaps indices into a per-core compacted table (int16-safe), the bf16
table is DMA'd into SBUF once, and all 8 mailbox slots are fetched with
SBUF-source transposing dma_gather, which lands rows feature-major [D, n] --
exactly the matmul rhs layout (no on-chip transposes).  ln2 affine is folded
into the fc weights on host.
"""
import os
import sys
import math
import numpy as np

sys.path.insert(0, "/opt/trn_rl_repo")

import ml_dtypes

BF16 = ml_dtypes.bfloat16

D = 128
N_SRC = 120000
N_DST = 50000
L = 8
N_CLASSES = 104
EPS = 1e-5
N_CORES = 8

ND = N_DST // N_CORES          # 6250 dst rows per core
NDP = 6272                     # padded to 49 cols of 128
# column groups for compute: 12 groups of 4 cols (512 dst) + 1 group of 1 col
GROUPS = [(g * 4, 4) for g in range(12)] + [(48, 1)]
NIDXC = NDP * L // 16          # 3136 idx columns in the [16, .] wrapped layout

_CACHE = {}


def _idx_col_base(gi, l):
    """Column offset of (group, slot) block in the wrapped idx layout."""
    if gi < 12:
        return gi * 256 + l * 32
    return 12 * 256 + l * 8


def _build_nc(nranks, reps, nq=4):
    # Non-transpose HBM-source gathers are safe across all 4 SWDGE queues
    # (the baseline kernel shipped this mode at nq=4).  Transpose-mode
    # SBUF-source gathers intermittently corrupt 16-token write bursts when
    # run on >1 queue concurrently (stress_probe.py), which is why this
    # kernel transposes on the tensor engine instead.
    import concourse.bass as bass
    import concourse.tile as tile
    from concourse import bacc, mybir

    fp32 = mybir.dt.float32
    bf16 = mybir.dt.bfloat16
    i16 = mybir.dt.int16
    AF = mybir.ActivationFunctionType
    ALU = mybir.AluOpType

    nc = bacc.Bacc(None, num_swdge_queues=4)

    TW = nranks * 128  # table free-dim in bf16 elements

    tabr = nc.declare_dram_parameter("tabr", [TW, 128], bf16, isOutput=False)
    idxw = nc.declare_dram_parameter("idxw", [16, NIDXC], i16, isOutput=False)
    wts = nc.declare_dram_parameter("wts", [128, 6 * 128], bf16, isOutput=False)
    fcwT = nc.declare_dram_parameter("fcwT", [128, N_CLASSES], bf16, isOutput=False)
    vecs = nc.declare_dram_parameter("vecs", [128, 8], fp32, isOutput=False)
    onesm = nc.declare_dram_parameter("onesm", [128, 128], bf16, isOutput=False)
    ident = nc.declare_dram_parameter("ident", [128, 128], bf16, isOutput=False)
    out = nc.declare_dram_parameter("out", [N_CLASSES, NDP], fp32, isOutput=True)

    with tile.TileContext(nc) as tc:
        with (
            tc.tile_pool(name="const", bufs=1) as cpool,
            tc.tile_pool(name="gx", bufs=3) as gpool,
            tc.tile_pool(name="work", bufs=2) as wpool,
            tc.tile_pool(name="outp", bufs=2) as opool,
            tc.tile_pool(name="ps", bufs=1, space=bass.MemorySpace.PSUM) as pspool,
        ):
            reg512 = nc.gpsimd.to_reg(512)
            reg128 = nc.gpsimd.to_reg(128)

            def body():
                # ---- load constants ----
                wt = cpool.tile([128, 6 * 128], bf16, tag="wt")
                nc.sync.dma_start(out=wt[:], in_=wts[:])
                fcw = cpool.tile([128, N_CLASSES], bf16, tag="fcw")
                nc.sync.dma_start(out=fcw[:], in_=fcwT[:])
                vec = cpool.tile([128, 8], fp32, tag="vec")
                nc.sync.dma_start(out=vec[:], in_=vecs[:])
                ones_t = cpool.tile([128, 128], bf16, tag="ones")
                nc.sync.dma_start(out=ones_t[:], in_=onesm[:])
                id_t = cpool.tile([128, 128], bf16, tag="ident")
                nc.sync.dma_start(out=id_t[:], in_=ident[:])
                idxt = cpool.tile([128, NIDXC], i16, tag="idx")
                for k in range(8):  # replicate [16, .] across the 8 quad-groups
                    nc.sync.dma_start(out=idxt[16 * k:16 * (k + 1), :], in_=idxw[:])

                w_ix, w_ih = wt[:, 0:128], wt[:, 128:256]
                w_ox, w_oh = wt[:, 256:384], wt[:, 384:512]
                w_ux, w_uh = wt[:, 512:640], wt[:, 640:768]
                bi, bo, bu = vec[:, 0:1], vec[:, 1:2], vec[:, 2:3]
                eps = vec[:, 3:4]
                fcb = vec[:N_CLASSES, 4:5]

                qn = 0
                for gi, (c0, ncols) in enumerate(GROUPS):
                    n = ncols * 128

                    # token-major gathered rows: gt[:, l, j, :] holds slot l,
                    # dst chunk j (tokens j*128..j*128+127 of this group)
                    gt = gpool.tile([128, L, 4, 128], bf16, tag="gt")
                    for l in range(L):
                        cb = _idx_col_base(gi, l)
                        nc.gpsimd.dma_gather(
                            out_ap=gt[:, l, :ncols, :], in_ap=tabr[:],
                            idxs_ap=idxt[:, cb:cb + n // 16],
                            num_idxs=n,
                            num_idxs_reg=reg512 if n == 512 else reg128,
                            elem_size=D,
                            queue_num=qn % nq)
                        qn += 1

                    # ---- transpose to feature-major on PE; h_sum by PSUM
                    # accumulation over the 7 child slots ----
                    ps_h = pspool.tile([128, 512], fp32, tag="ps_h")
                    ps_x = pspool.tile([128, 512], fp32, tag="ps_x")
                    for j in range(ncols):
                        jc = slice(j * 128, (j + 1) * 128)
                        for l in range(L - 1):
                            nc.tensor.matmul(ps_h[:, jc], gt[:, l, j, :],
                                             id_t[:], start=(l == 0),
                                             stop=(l == L - 2))
                        nc.tensor.matmul(ps_x[:, jc], gt[:, L - 1, j, :],
                                         id_t[:], start=True, stop=True)
                    hs = wpool.tile([128, 512], bf16, tag="hs")
                    xt = wpool.tile([128, 512], bf16, tag="xt")
                    nc.vector.tensor_copy(out=hs[:, :n], in_=ps_h[:, :n])
                    nc.vector.tensor_copy(out=xt[:, :n], in_=ps_x[:, :n])
                    xg = xt[:, :n]  # self message, feature-major

                    # ---- gates: psum = Wx.T@x + Wh.T@h_sum ----
                    ps_i = pspool.tile([128, 512], fp32, tag="ps_i")
                    ps_o = pspool.tile([128, 512], fp32, tag="ps_o")
                    ps_u = pspool.tile([128, 512], fp32, tag="ps_u")
                    for ps, wx, wh in ((ps_i, w_ix, w_ih), (ps_o, w_ox, w_oh),
                                       (ps_u, w_ux, w_uh)):
                        nc.tensor.matmul(ps[:, :n], wx, xg,
                                         start=True, stop=False)
                        nc.tensor.matmul(ps[:, :n], wh, hs[:, :n],
                                         start=False, stop=True)

                    ig = wpool.tile([128, 512], bf16, tag="ig")
                    og = wpool.tile([128, 512], bf16, tag="og")
                    ug = wpool.tile([128, 512], bf16, tag="ug")
                    hg = wpool.tile([128, 512], bf16, tag="hg")
                    nc.scalar.activation(out=ig[:, :n], in_=ps_i[:, :n],
                                         func=AF.Sigmoid, bias=bi)
                    nc.scalar.activation(out=og[:, :n], in_=ps_o[:, :n],
                                         func=AF.Sigmoid, bias=bo)
                    nc.scalar.activation(out=ug[:, :n], in_=ps_u[:, :n],
                                         func=AF.Tanh, bias=bu)
                    # c = i*u (into ug), t = tanh(c) (into ig), h = o*t
                    nc.vector.tensor_tensor(out=ug[:, :n], in0=ig[:, :n],
                                            in1=ug[:, :n], op=ALU.mult)
                    nc.scalar.activation(out=ig[:, :n], in_=ug[:, :n],
                                         func=AF.Tanh)
                    nc.vector.tensor_tensor(out=hg[:, :n], in0=og[:, :n],
                                            in1=ig[:, :n], op=ALU.mult)

                    # ---- LayerNorm over features (= partitions) ----
                    sq = wpool.tile([128, 512], bf16, tag="sq")
                    nc.vector.tensor_tensor(out=sq[:, :n], in0=hg[:, :n],
                                            in1=hg[:, :n], op=ALU.mult)
                    mu_b = pspool.tile([128, 512], fp32, tag="mu_b")
                    ms_b = pspool.tile([128, 512], fp32, tag="ms_b")
                    nc.tensor.matmul(mu_b[:, :n], ones_t[:], hg[:, :n],
                                     start=True, stop=True)
                    nc.tensor.matmul(ms_b[:, :n], ones_t[:], sq[:, :n],
                                     start=True, stop=True)
                    var = wpool.tile([128, 512], fp32, tag="var")
                    nc.scalar.activation(out=var[:, :n], in_=mu_b[:, :n],
                                         func=AF.Square)
                    nc.vector.tensor_tensor(out=var[:, :n], in0=ms_b[:, :n],
                                            in1=var[:, :n], op=ALU.subtract)
                    # rstd = 1/sqrt(var + eps)
                    nc.scalar.activation(out=var[:, :n], in_=var[:, :n],
                                         func=AF.Sqrt, bias=eps)
                    nc.vector.reciprocal(out=var[:, :n], in_=var[:, :n])
                    # hn = (h - mu) * rstd   (ln2 affine folded into fc)
                    nc.vector.tensor_tensor(out=hg[:, :n], in0=hg[:, :n],
                                            in1=mu_b[:, :n], op=ALU.subtract)
                    nc.vector.tensor_tensor(out=hg[:, :n], in0=hg[:, :n],
                                            in1=var[:, :n], op=ALU.mult)

                    # ---- fc head: logits.T [104, n] ----
                    fcp = pspool.tile([N_CLASSES, 512], fp32, tag="fcp")
                    nc.tensor.matmul(fcp[:, :n], fcw[:], hg[:, :n],
                                     start=True, stop=True)
                    lg = opool.tile([N_CLASSES, 512], fp32, tag="lg")
                    nc.scalar.activation(out=lg[:, :n], in_=fcp[:, :n],
                                         func=AF.Identity, bias=fcb)
                    nc.sync.dma_start(out=out[:, c0 * 128: c0 * 128 + n],
                                      in_=lg[:, :n])

            if reps == 1:
                body()
            else:
                with tc.For_i(0, reps):
                    body()

    _fix_swdge(nc, nq)
    nc.finalize()
    return nc


def _fix_swdge(nc, nq):
    """Post-build SWDGE queue/semaphore alignment.

    1. Route every gather to queue (DMASW lane % nq), matching the
       Tile-assigned completion-sem lane.
    2. The For_i reset/skip blocks bulk-adjust all 8 DMASW sems with a single
       queue-0 InstIncSwdgeSem; each sem is locked to the queue its gathers
       use, so split the adjustment into one instruction per queue.
    """
    from concourse import bass_isa, mybir
    DMASW0 = 11
    for b in nc.m.functions[0].blocks:
        for inst in b.instructions:
            if isinstance(inst, mybir.InstDMAGatherAnt):
                inst.queue_num = (inst.bass_scheduled_proc - DMASW0) % nq
    for b in nc.m.functions[0].blocks:
        new_insts = []
        for inst in b.instructions:
            new_insts.append(inst)
            if not (isinstance(inst, bass_isa.InstIncSwdgeSem)
                    and inst._mode in ("add", "sub")):
                continue
            names = list(inst._sem_names)
            vals = list(inst._sem_values)
            lanes = [i for i, (nm, v) in enumerate(zip(names, vals))
                     if v and nm.startswith("DMASW")]
            qs = sorted({i % 4 for i in lanes})
            if len(qs) <= 1:
                if lanes:
                    inst.queue_num = qs[0]
                continue

            def masked(q):
                return [v if (i % 4 == q and i in lanes) else 0
                        for i, v in enumerate(vals)]

            inst._sem_values = masked(qs[0])
            inst.queue_num = qs[0]
            for q in qs[1:]:
                clone = bass_isa.InstIncSwdgeSem(
                    _sem_id_base=inst._sem_id_base,
                    _sem_values=masked(q),
                    _sem_names=names,
                    queue_num=q,
                    _mode=inst._mode,
                    name=nc.get_next_instruction_name(),
                    engine=inst.engine,
                )
                nc.register_instruction(clone)
                new_insts.append(clone)
        b.instructions[:] = new_insts


def get_nc(nranks, reps=1):
    key = ("nc", nranks, reps)
    if key not in _CACHE:
        _CACHE[key] = _build_nc(nranks, reps)
    return _CACHE[key]


def _prep_host(inputs):
    token_ids = np.asarray(inputs["token_ids"]).astype(np.int64)
    mailbox_idx = np.asarray(inputs["mailbox_idx"]).astype(np.int64)
    emb = np.asarray(inputs["emb"], dtype=np.float32)

    idx2 = token_ids[mailbox_idx]  # [N_DST, L]

    uniqs, invs = [], []
    for c in range(N_CORES):
        rows = idx2[c * ND:(c + 1) * ND]
        uniq, inv = np.unique(rows, return_inverse=True)
        assert uniq.size <= 32767, f"core {c}: {uniq.size} unique rows > int16"
        uniqs.append(uniq)
        invs.append(inv.reshape(ND, L))
    nranks = (max(u.size for u in uniqs) + 127) // 128

    # weights: ln2 affine folded into the fc head
    ln2_g = np.asarray(inputs["ln2_g"], np.float32)
    ln2_b = np.asarray(inputs["ln2_b"], np.float32)
    fc_w = np.asarray(inputs["fc_w"], np.float32)
    wts = np.concatenate(
        [np.ascontiguousarray(np.asarray(inputs[k], np.float32).T) for k in
         ("ix_w", "ih_w", "ox_w", "oh_w", "ux_w", "uh_w")],
        axis=1).astype(BF16)  # [128, 768]
    fcwT = np.ascontiguousarray((fc_w * ln2_g[None, :]).T).astype(BF16)
    vecs = np.zeros((128, 8), np.float32)
    vecs[:, 0] = np.asarray(inputs["ix_b"]) + np.asarray(inputs["ih_b"])
    vecs[:, 1] = np.asarray(inputs["ox_b"]) + np.asarray(inputs["oh_b"])
    vecs[:, 2] = np.asarray(inputs["ux_b"]) + np.asarray(inputs["uh_b"])
    vecs[:, 3] = EPS
    vecs[:N_CLASSES, 4] = np.asarray(inputs["fc_b"], np.float32) + fc_w @ ln2_b
    onesm = np.full((128, 128), 1.0 / D, np.float32).astype(BF16)
    ident = np.eye(128, dtype=np.float32).astype(BF16)

    shared = dict(wts=wts, fcwT=fcwT, vecs=vecs, onesm=onesm, ident=ident)
    in_maps = []
    for c in range(N_CORES):
        uniq, inv = uniqs[c], invs[c]
        tw = nranks * 128
        tabr = np.zeros((tw, D), BF16)
        tabr[:uniq.size] = emb[uniq].astype(BF16)

        inv2 = np.zeros((NDP, L), np.int64)
        inv2[:ND] = inv
        Y = inv2[:6144].reshape(12, 32, 16, L).transpose(2, 0, 3, 1).reshape(16, -1)
        Z = inv2[6144:].reshape(8, 16, L).transpose(1, 2, 0).reshape(16, -1)
        idxw = np.concatenate([Y, Z], axis=1).astype(np.int16)  # [16, 3136]

        m = dict(shared)
        m["tabr"] = tabr
        m["idxw"] = idxw
        in_maps.append(m)
    return in_maps, nranks


def kernel(**inputs):
    from concourse.bass_utils import run_bass_kernel_spmd

    in_maps, nranks = _prep_host(inputs)
    _CACHE["in_maps"] = in_maps
    _CACHE["nranks"] = nranks
    nc = get_nc(nranks, reps=1)

    res = run_bass_kernel_spmd(nc, in_maps, list(range(N_CORES)))
    _CACHE["last_results"] = res

    out = np.empty((N_DST, N_CLASSES), np.float32)
    for c in range(N_CORES):
        out[c * ND:(c + 1) * ND] = res.results[c]["out"][:, :ND].T
    return out


# ---------------------------------------------------------------------------
# Timing: HW exec time of one kernel execution, measured as the marginal cost
# per iteration of an on-device For_i reps-loop (cancels the axon RPC
# dispatch overhead which would otherwise swamp the kernel by ~500x).
# Inputs are staged on device once; the same jitted executable is reused.
# ---------------------------------------------------------------------------

def _make_runner(nc, in_maps):
    import jax
    import numpy as np
    from jax.sharding import Mesh, PartitionSpec, NamedSharding
    from jax.experimental.shard_map import shard_map
    from concourse import bass2jax, mybir
    bass2jax.install_neuronx_cc_hook()

    partition_name = (nc.partition_id_tensor.name
                      if nc.partition_id_tensor else None)
    in_names, out_names, out_avals, zero_outs = [], [], [], []
    for alloc in nc.m.functions[0].allocations:
        if not isinstance(alloc, mybir.MemoryLocationSet):
            continue
        name = alloc.memorylocations[0].name
        if alloc.kind == "ExternalInput":
            if name != partition_name:
                in_names.append(name)
        elif alloc.kind == "ExternalOutput":
            out_names.append(name)
            shape = tuple(alloc.tensor_shape)
            dtype = mybir.dt.np(alloc.dtype)
            out_avals.append(jax.core.ShapedArray(shape, dtype))
            zero_outs.append(np.zeros(shape, dtype))
    n_params = len(in_names)
    n_outs = len(out_avals)
    all_names = in_names + out_names
    if partition_name is not None:
        all_names = all_names + [partition_name]

    def _body(*args):
        operands = list(args)
        if partition_name is not None:
            operands.append(bass2jax.partition_id_tensor())
        outs = bass2jax._bass_exec_p.bind(
            *operands,
            out_avals=tuple(out_avals),
            in_names=tuple(all_names),
            out_names=tuple(out_names),
            lowering_input_output_aliases=(),
            sim_require_finite=True,
            sim_require_nnan=True,
            nc=nc,
        )
        return tuple(outs)

    devices = jax.devices()[:N_CORES]
    mesh = Mesh(np.asarray(devices), ("core",))
    spec = PartitionSpec("core")
    sharded = jax.jit(
        shard_map(_body, mesh=mesh, in_specs=(spec,) * (n_params + n_outs),
                  out_specs=(spec,) * n_outs, check_rep=False),
        donate_argnums=tuple(range(n_params, n_params + n_outs)),
        keep_unused=True,
    )
    sharding = NamedSharding(mesh, spec)
    dkey = ("dev_in", tuple(in_names))
    if dkey not in _CACHE:
        dev_in = [
            jax.device_put(
                np.concatenate([np.asarray(in_maps[c][nm])
                                for c in range(N_CORES)], axis=0), sharding)
            for nm in in_names
        ]
        for a in dev_in:
            a.block_until_ready()
        _CACHE[dkey] = dev_in
    dev_in = _CACHE[dkey]

    def run():
        zs = [jax.device_put(
            np.zeros((N_CORES * z.shape[0], *z.shape[1:]), z.dtype), sharding)
            for z in zero_outs]
        for z in zs:
            z.block_until_ready()
        import time
        t0 = time.perf_counter()
        outs = sharded(*dev_in, *zs)
        jax.block_until_ready(outs)
        return time.perf_counter() - t0, outs

    return run


def measure_hw_exec_ns(reps=1024, n_meas=5, verbose=True):
    """Marginal per-iteration HW time: (t[reps] - t[1]) / (reps - 1)."""
    in_maps, nranks = _CACHE["in_maps"], _CACHE["nranks"]
    if ("runner", 1) not in _CACHE:
        _CACHE[("runner", 1)] = _make_runner(get_nc(nranks, 1), in_maps)
    if ("runner", reps) not in _CACHE:
        _CACHE[("runner", reps)] = _make_runner(get_nc(nranks, reps), in_maps)
    r1, rk = _CACHE[("runner", 1)], _CACHE[("runner", reps)]
    # warm both executables; cross-check the staged-input path against the
    # correctness run
    _, outs1 = r1()
    ref_res = _CACHE.get("last_results")
    if ref_res is not None:
        got = np.asarray(outs1[0]).reshape(N_CORES, N_CLASSES, NDP)
        for c in range(N_CORES):
            assert np.allclose(got[c], ref_res.results[c]["out"],
                               rtol=1e-3, atol=1e-3), f"runner mismatch core {c}"
    _, outsk = rk()
    gotk = np.asarray(outsk[0]).reshape(N_CORES, N_CLASSES, NDP)
    assert np.allclose(gotk, np.asarray(outs1[0]).reshape(gotk.shape),
                       rtol=1e-3, atol=1e-3), "reps-loop output mismatch"
    t1s, tks = [], []
    for _ in range(n_meas):
        t1s.append(r1()[0])
        tks.append(rk()[0])
    t1s.sort(); tks.sort()
    med1 = t1s[len(t1s) // 2]
    medk = tks[len(tks) // 2]
    if verbose:
        print(f"t[1] runs: {[f'{t*1e3:.1f}ms' for t in t1s]}")
        print(f"t[{reps}] runs: {[f'{t*1e3:.1f}ms' for t in tks]}")
    return (medk - med1) / (reps - 1) * 1e9
